# revision 34
# baseline (speedup 1.0000x reference)
"""Trainium2 Bass kernel for a dense transformer block (B=2, T=2048, C=1024, H=16).

Sharding v3: (batch, head-group) tensor-parallel attention across 8 cores
(core = one batch x 4 heads), 8-rank AllToAll with cross-batch 256-token
strips, then row-parallel FFN (512 tokens/core). Feature-major dataflow off
a host-transposed x^T; LN stats via ones-matmuls; fp8 DoubleRow matmuls for
QKV / Wo / FFN with host-scaled weights; fp8 A2A payload. Output y^T is
un-transposed on the host.
"""

import numpy as np
import ml_dtypes

import concourse.bass as bass
import concourse.bacc as bacc
import concourse.mybir as mybir
import concourse.tile as tile
from concourse.masks import make_identity


F32 = mybir.dt.float32
BF16 = mybir.dt.bfloat16
F8 = mybir.dt.float8e4
AF = mybir.ActivationFunctionType
ALU = mybir.AluOpType
DR = mybir.MatmulPerfMode.DoubleRow

N_CORES = 8
NG = 4                  # cores per group (one batch per group)
B, T, C, H, D, FF = 2, 2048, 1024, 16, 64, 4096
HPC = H // NG           # 4 heads per core
FPC = HPC * D           # 256 features per core
KT = C // 128           # 8 k-tiles of embedding dim
CH = 512                # token chunk
NCH = T // CH           # 4 chunks per batch
SCALE = 1.0 / np.sqrt(C)
LN_EPS = 1e-5
SW = 2.0 ** 12          # fp8 weight scale (wq/wk/wv/wo/w1)
SW2 = 2.0 ** 13         # fp8 weight scale (w2)
ISW = 1.0 / SW
ISW2 = 1.0 / SW2
N_WARM = 64
N_DUMMY = 64


def build_nc():
    nc = bacc.Bacc(None, target_bir_lowering=False, debug=False, num_devices=N_CORES)

    # ---- per-core inputs (host pre-laid-out) ----
    xt = nc.dram_tensor("xt", [128, KT, T], BF16, kind="ExternalInput").ap()
    xself = nc.dram_tensor("xself", [128, KT, CH], BF16, kind="ExternalInput").ap()
    wq = nc.dram_tensor("wq", [128, KT, FPC], F8, kind="ExternalInput").ap()
    wk = nc.dram_tensor("wk", [128, KT, FPC], F8, kind="ExternalInput").ap()
    wv = nc.dram_tensor("wv", [128, KT, FPC], F8, kind="ExternalInput").ap()
    bqkv = nc.dram_tensor("bqkv", [128, 2, 3], F32, kind="ExternalInput").ap()
    wo = nc.dram_tensor("wo", [128, KT, KT, 128], F8, kind="ExternalInput").ap()
    bo_col = nc.dram_tensor("bo_col", [128, KT], F32, kind="ExternalInput").ap()
    w1 = nc.dram_tensor("w1", [128, KT, FF], F8, kind="ExternalInput").ap()
    b1 = nc.dram_tensor("b1", [128, 32], F32, kind="ExternalInput").ap()
    w2 = nc.dram_tensor("w2", [KT, 128, 32, 128], F8, kind="ExternalInput").ap()
    b2col = nc.dram_tensor("b2col", [128, KT], F32, kind="ExternalInput").ap()
    masks = nc.dram_tensor("masks", [128, 4, 2, CH], BF16, kind="ExternalInput").ap()
    y = nc.dram_tensor("y", [KT, 128, CH], F32, kind="ExternalOutput").ap()

    with tile.TileContext(nc) as tc:
        with (
            tc.tile_pool(name="const", bufs=1) as const,
            tc.tile_pool(name="dram", bufs=1, space="DRAM") as dram,
        ):
            ident = const.tile([128, 128], BF16)
            make_identity(nc, ident[:])
            ones_c = const.tile([128, 1], BF16)
            nc.any.memset(ones_c[:], 1.0 / C)
            ones_1 = const.tile([128, 1], BF16)
            nc.any.memset(ones_1[:], 1.0)
            ones64b = const.tile([128, 64], BF16)
            nc.any.memset(ones64b[:], 1.0)
            onesrow = const.tile([1, CH], BF16)
            nc.any.memset(onesrow[:], 1.0)
            eps1 = const.tile([1, 1], F32)
            nc.any.memset(eps1[:], LN_EPS)

            # A2A slots: dest core c' gets my 4 heads for a 256-token strip of
            # my batch (stage E rows: 256 from b0 + 256 from b1)
            a2a_in = dram.tile([N_CORES, 2, 128, CH // 2], F8)
            a2a_out = dram.tile([N_CORES, 2, 128, CH // 2], F8)

            # attention persistent tensors
            qkv_cm = tc.tile_pool(name="qkvp", bufs=1)
            qkvp = qkv_cm.__enter__()
            qt_sb = qkvp.tile([128, 2, T], BF16)
            kt_sb = qkvp.tile([128, 2, T], BF16)
            vt_sb = qkvp.tile([128, 2, T], BF16)
            v_sb = qkvp.tile([128, T // 128, FPC], BF16)
            masks_sb = qkvp.tile([128, 4, 2, CH], BF16)

            # ================= Phase 1: LN1 + QKV (feature-major) =================
            with (
                tc.tile_pool(name="p1", bufs=2) as p1,
                tc.tile_pool(name="p1s", bufs=3) as p1s,
                tc.tile_pool(name="ps1", bufs=3, space="PSUM") as ps1,
                tc.tile_pool(name="pstat", bufs=2, space="PSUM") as pstat,
            ):
                # x^T chunk loads first so the stats matmuls can start early
                xts = []
                for n in range(NCH):
                    xt_c = p1.tile([128, KT, CH], BF16, tag="xt", bufs=4,
                                   name=f"xtc{n}")
                    nc.sync.dma_start(xt_c[:], xt[:, :, CH * n:CH * (n + 1)])
                    xts.append(xt_c)
                # weights after the x^T stream
                wq_sb = const.tile([128, KT, FPC], F8, name="wq_sb")
                nc.sync.dma_start(wq_sb[:], wq[:])
                wk_sb = const.tile([128, KT, FPC], F8, name="wk_sb")
                nc.sync.dma_start(wk_sb[:], wk[:])
                wv_sb = const.tile([128, KT, FPC], F8, name="wv_sb")
                nc.sync.dma_start(wv_sb[:], wv[:])
                bqkv_sb = const.tile([128, 2, 3], F32, name="bqkv_sb")
                nc.sync.dma_start(bqkv_sb[:], bqkv[:])
                nc.sync.dma_start(masks_sb[:], masks[:])
                xself_sb = const.tile([128, KT, CH], BF16, name="xself_sb")
                nc.sync.dma_start(xself_sb[:], xself[:])
                wo_sb = const.tile([128, KT, KT, 128], F8, name="wo_sb")
                bo_sb = const.tile([128, KT], F32, name="bo_sb")
                nc.sync.dma_start(bo_sb[:], bo_col[:])
                b1_sb = const.tile([128, 32], F32, name="b1_sb")
                nc.sync.dma_start(b1_sb[:], b1[:])
                b2_sb = const.tile([128, KT], F32, name="b2_sb")
                nc.sync.dma_start(b2_sb[:], b2col[:])

                # HAM warmup: PE activity with no DMA dependency
                ps_w = ps1.tile([128, CH], F32, tag="warm", bufs=1)
                for wi in range(N_WARM):
                    nc.tensor.matmul(ps_w[:, 0:128], lhsT=ident[:], rhs=ident[:],
                                     start=(wi == 0), stop=(wi == N_WARM - 1))

                for n in range(NCH):
                    q0 = CH * n
                    xt_c = xts[n]
                    st = pstat.tile([128, CH], F32, tag="stat")
                    for k in range(KT):
                        nc.tensor.matmul(st[0:1, :], lhsT=ones_c[:], rhs=xt_c[:, k, :],
                                         start=(k == 0), stop=(k == KT - 1))
                    # E[x^2] from raw x (independent of mu -> shorter chain)
                    for k in range(KT):
                        sq = p1s.tile([128, CH], BF16, tag="sq", bufs=3)
                        eng = nc.gpsimd if k % 2 == 0 else nc.vector
                        eng.tensor_tensor(out=sq[:], in0=xt_c[:, k, :],
                                          in1=xt_c[:, k, :], op=ALU.mult)
                        nc.tensor.matmul(st[32:33, :], lhsT=ones_c[:], rhs=sq[:],
                                         start=(k == 0), stop=(k == KT - 1))
                    mur = p1s.tile([1, CH], BF16, tag="mur")
                    nc.scalar.copy(out=mur[:], in_=st[0:1, :])
                    murf = p1s.tile([1, CH], F32, tag="murf")
                    nc.scalar.copy(out=murf[:], in_=st[0:1, :])
                    psb = ps1.tile([128, CH], F32, tag="psmm")
                    nc.tensor.matmul(psb[:], lhsT=onesrow[0:1, 0:128], rhs=mur[:],
                                     start=True, stop=True)
                    mub = p1s.tile([128, CH], BF16, tag="mub")
                    nc.scalar.copy(out=mub[:], in_=psb[:])
                    musq = p1s.tile([1, CH], F32, tag="musq")
                    nc.vector.tensor_tensor(out=musq[:], in0=murf[:], in1=murf[:],
                                            op=ALU.mult)
                    varr = p1s.tile([1, CH], F32, tag="varr")
                    nc.vector.tensor_tensor(out=varr[:], in0=st[32:33, :], in1=musq[:],
                                            op=ALU.subtract)
                    stdr = p1s.tile([1, CH], F32, tag="stdr")
                    nc.scalar.activation(out=stdr[:], in_=varr[:], func=AF.Sqrt,
                                         bias=eps1[:], scale=1.0)
                    rstdrf = p1s.tile([1, CH], F32, tag="rstdrf")
                    nc.vector.reciprocal_approx_fast(out=rstdrf[:], in_=stdr[:])
                    rstdr = p1s.tile([1, CH], BF16, tag="rstdr")
                    nc.scalar.copy(out=rstdr[:], in_=rstdrf[:])
                    psb2 = ps1.tile([128, CH], F32, tag="psmm")
                    nc.tensor.matmul(psb2[:], lhsT=onesrow[0:1, 0:128], rhs=rstdr[:],
                                     start=True, stop=True)
                    rstdb = p1s.tile([128, CH], BF16, tag="rstdb")
                    nc.scalar.copy(out=rstdb[:], in_=psb2[:])
                    xc = p1.tile([128, KT, CH], BF16, tag="xc")
                    for k in range(KT):
                        nc.vector.tensor_tensor(out=xc[:, k, :], in0=xt_c[:, k, :],
                                                in1=mub[:], op=ALU.subtract)
                    h_c = p1.tile([128, KT, CH], F8, tag="h")
                    for k in range(KT):
                        nc.vector.tensor_tensor(out=h_c[:, k, :], in0=xc[:, k, :],
                                                in1=rstdb[:], op=ALU.mult)
                    # QKV matmuls: fp8 DoubleRow, weights pre-scaled by SW
                    for w_sb, out_sb, col in ((wq_sb, qt_sb, 0), (wk_sb, kt_sb, 1),
                                              (wv_sb, vt_sb, 2)):
                        for g in range(2):
                            ps = ps1.tile([128, CH], F32, tag="psmm")
                            for t2 in range(KT // 2):
                                nc.tensor.matmul(
                                    ps[:],
                                    lhsT=w_sb[:, 2 * t2:2 * t2 + 2, 128 * g:128 * (g + 1)],
                                    rhs=h_c[:, 2 * t2:2 * t2 + 2, :],
                                    start=(t2 == 0), stop=(t2 == KT // 2 - 1),
                                    perf_mode=DR)
                            nc.vector.tensor_scalar(
                                out=out_sb[:, g, q0:q0 + CH], in0=ps[:],
                                scalar1=ISW, scalar2=bqkv_sb[:, g, col:col + 1],
                                op0=ALU.mult, op1=ALU.add)
                    # V -> token-major for this chunk (PE transposes)
                    for g in range(2):
                        ps_t = ps1.tile([128, CH], BF16, tag="psmm")
                        for u in range(4):
                            nc.tensor.transpose(
                                ps_t[:, 128 * u:128 * (u + 1)],
                                vt_sb[:, g, q0 + 128 * u:q0 + 128 * (u + 1)], ident[:])
                        nc.scalar.copy(
                            out=v_sb[:, 4 * n:4 * n + 4, 128 * g:128 * (g + 1)],
                            in_=ps_t[:].rearrange("p (a b) -> p a b", a=4))
                nc.sync.dma_start(wo_sb[:], wo[:])

            # ================= Phase 2: attention (S^T orientation) =================
            with (
                tc.tile_pool(name="pss", bufs=1, space="PSUM") as pssp,
                tc.tile_pool(name="pap", bufs=1, space="PSUM") as pap,
                tc.tile_pool(name="psr", bufs=1, space="PSUM") as psr,
                tc.tile_pool(name="ptp", bufs=9) as ptp,
                tc.tile_pool(name="smp", bufs=2) as smp,
            ):
                for qc in range(NCH):
                    q0 = CH * qc
                    nkt = 4 * (qc + 1)
                    pa0 = pap.tile([128, CH], F32, tag="pa0", name="pa0")
                    pa1 = pap.tile([128, CH], F32, tag="pa1", name="pa1")
                    pa = [pa0, pa1]
                    pasum = pap.tile([128, CH], F32, tag="pasum")
                    pts = {}

                    def emit_qk(k):
                        d = k - 4 * qc
                        qlo = 128 * d if d > 0 else 0
                        for gg in range(2):
                            ps = pssp.tile([128, 2, CH], F32, tag=f"pss{gg}")
                            for hh in range(2):
                                hp = 64 * hh
                                nc.tensor.matmul(
                                    ps[:, hh, qlo:],
                                    lhsT=kt_sb[hp:hp + 64, gg, 128 * k:128 * (k + 1)],
                                    rhs=qt_sb[hp:hp + 64, gg, q0 + qlo:q0 + CH],
                                    start=True, stop=True, tile_position=(hp, 0))
                            pt = ptp.tile([128, 2, CH], BF16, tag="pt")
                            nc.scalar.activation(out=pt[:, :, qlo:], in_=ps[:, :, qlo:],
                                                 func=AF.Exp, scale=SCALE)
                            if d >= 0:
                                nc.gpsimd.tensor_tensor(
                                    out=pt[:, :, qlo:], in0=pt[:, :, qlo:],
                                    in1=masks_sb[:, d, :, qlo:], op=ALU.mult)
                            pts[(k, gg)] = pt

                    def emit_pv(k):
                        d = k - 4 * qc
                        qlo = 128 * d if d > 0 else 0
                        for gg in range(2):
                            pt = pts.pop((k, gg))
                            for hh in range(2):
                                nc.tensor.matmul(
                                    pa[gg][64 * hh:64 * (hh + 1), qlo:],
                                    lhsT=v_sb[:, k, 128 * gg + 64 * hh:128 * gg + 64 * (hh + 1)],
                                    rhs=pt[:, hh, qlo:],
                                    start=(k == 0), stop=(k == nkt - 1),
                                    tile_position=(0, 64 * hh),
                                    skip_group_check=(hh == 1))
                            for hh in range(2):
                                h = 2 * gg + hh
                                nc.tensor.matmul(
                                    pasum[32 * h:32 * h + 1, qlo:],
                                    lhsT=ones_1[:], rhs=pt[:, hh, qlo:],
                                    start=(k == 0), stop=(k == nkt - 1),
                                    tile_position=(0, 32 * h),
                                    skip_group_check=(h > 0))

                    for k in range(nkt + 3):
                        if k < nkt:
                            emit_qk(k)
                        if k >= 3:
                            emit_pv(k - 3)

                    recf = smp.tile([128, CH], F32, tag="recf")
                    nc.vector.reciprocal_approx_fast(out=recf[:], in_=pasum[:])
                    rec = smp.tile([128, CH], BF16, tag="rec")
                    nc.vector.tensor_scalar(out=rec[:], in0=recf[:], scalar1=1.0,
                                            scalar2=None, op0=ALU.mult)
                    for gg in range(2):
                        an = smp.tile([128, CH], BF16, tag=f"an{gg}")
                        nc.scalar.copy(out=an[:], in_=pa[gg][:])
                        rb = psr.tile([128, CH], F32, tag="recb")
                        for hh in range(2):
                            h = 2 * gg + hh
                            nc.tensor.matmul(
                                rb[64 * hh:64 * (hh + 1), :],
                                lhsT=ones64b[32 * h:32 * h + 1, :],
                                rhs=rec[32 * h:32 * h + 1, :],
                                start=True, stop=True,
                                tile_position=(32 * h, 64 * hh),
                                skip_group_check=(hh == 1))
                        at_t = smp.tile([128, CH], F8, tag=f"at{gg}")
                        nc.vector.tensor_tensor(out=at_t[:], in0=an[:], in1=rb[:],
                                                op=ALU.mult)
                        nc.sync.dma_start(out=a2a_in[2 * qc, gg],
                                          in_=at_t[:, 0:CH // 2])
                        nc.sync.dma_start(out=a2a_in[2 * qc + 1, gg],
                                          in_=at_t[:, CH // 2:CH])

            qkv_cm.__exit__(None, None, None)

            # ================= Phase 3: AllToAll (8 ranks, fp8 payload) ============
            nc.gpsimd.collective_compute(
                "AllToAll", ALU.bypass,
                replica_groups=[list(range(N_CORES))],
                ins=[a2a_in[:].opt()], outs=[a2a_out[:].opt()],
            )

            # ================= Phase 4: Wo + LN2 + FFN (feature-major) ============
            with (
                tc.tile_pool(name="ef", bufs=1) as ef,
                tc.tile_pool(name="efw", bufs=2) as efw,
                tc.tile_pool(name="psE", bufs=3, space="PSUM") as psE,
                tc.tile_pool(name="psES", bufs=1, space="PSUM") as psES,
                tc.tile_pool(name="w1p", bufs=4) as w1p,
                tc.tile_pool(name="w2p", bufs=3) as w2p,
            ):
                # keep-warm dummies riding over the collective
                dm = psES.tile([128, CH], F32, tag="dummy")
                for i in range(N_DUMMY):
                    nc.tensor.matmul(dm[:], lhsT=ident[:],
                                     rhs=xself_sb[:, 0, :],
                                     start=(i == 0), stop=(i == N_DUMMY - 1))

                # token axis of stage E: [0:256] = batch-0 strip, [256:512] = batch-1
                attnt = ef.tile([128, KT, CH], F8)
                for s in range(N_CORES):
                    bs, hgs = s // NG, s % NG
                    for g in range(2):
                        nc.sync.dma_start(
                            out=attnt[:, 2 * hgs + g,
                                      (CH // 2) * bs:(CH // 2) * (bs + 1)],
                            in_=a2a_out[s, g])

                # Wo (fp8 DoubleRow) + bo + residual
                x2 = ef.tile([128, KT, CH], BF16)
                for co in range(KT):
                    ps = psE.tile([128, CH], F32, tag="ps")
                    for t2 in range(KT // 2):
                        nc.tensor.matmul(ps[:],
                                         lhsT=wo_sb[:, 2 * t2:2 * t2 + 2, co, :],
                                         rhs=attnt[:, 2 * t2:2 * t2 + 2, :],
                                         start=(t2 == 0), stop=(t2 == KT // 2 - 1),
                                         perf_mode=DR)
                    prj = efw.tile([128, CH], BF16, tag="prj")
                    nc.scalar.activation(out=prj[:], in_=ps[:], func=AF.Identity,
                                         bias=bo_sb[:, co:co + 1], scale=ISW)
                    nc.vector.tensor_tensor(out=x2[:, co, :], in0=prj[:],
                                            in1=xself_sb[:, co, :], op=ALU.add)

                # LN2 (feature-major stats, E[x^2] form)
                st2 = psES.tile([128, CH], F32, tag="stat2")
                for k in range(KT):
                    nc.tensor.matmul(st2[0:1, :], lhsT=ones_c[:], rhs=x2[:, k, :],
                                     start=(k == 0), stop=(k == KT - 1))
                for k in range(KT):
                    sq2 = efw.tile([128, CH], BF16, tag="sq2", bufs=3)
                    nc.vector.tensor_tensor(out=sq2[:], in0=x2[:, k, :],
                                            in1=x2[:, k, :], op=ALU.mult)
                    nc.tensor.matmul(st2[32:33, :], lhsT=ones_c[:], rhs=sq2[:],
                                     start=(k == 0), stop=(k == KT - 1))
                mur2 = efw.tile([1, CH], BF16, tag="mur2")
                nc.scalar.copy(out=mur2[:], in_=st2[0:1, :])
                murf2 = efw.tile([1, CH], F32, tag="murf2")
                nc.scalar.copy(out=murf2[:], in_=st2[0:1, :])
                psb3 = psE.tile([128, CH], F32, tag="ps")
                nc.tensor.matmul(psb3[:], lhsT=onesrow[0:1, 0:128], rhs=mur2[:],
                                 start=True, stop=True)
                mub2 = efw.tile([128, CH], BF16, tag="mub2")
                nc.scalar.copy(out=mub2[:], in_=psb3[:])
                musq2 = efw.tile([1, CH], F32, tag="musq2")
                nc.vector.tensor_tensor(out=musq2[:], in0=murf2[:], in1=murf2[:],
                                        op=ALU.mult)
                varr2 = efw.tile([1, CH], F32, tag="varr2")
                nc.vector.tensor_tensor(out=varr2[:], in0=st2[32:33, :], in1=musq2[:],
                                        op=ALU.subtract)
                stdr2 = efw.tile([1, CH], F32, tag="stdr2")
                nc.scalar.activation(out=stdr2[:], in_=varr2[:], func=AF.Sqrt,
                                     bias=eps1[:], scale=1.0)
                rstdrf2 = efw.tile([1, CH], F32, tag="rstdrf2")
                nc.vector.reciprocal_approx_fast(out=rstdrf2[:], in_=stdr2[:])
                rstdr2 = efw.tile([1, CH], BF16, tag="rstdr2")
                nc.scalar.copy(out=rstdr2[:], in_=rstdrf2[:])
                psb4 = psE.tile([128, CH], F32, tag="ps")
                nc.tensor.matmul(psb4[:], lhsT=onesrow[0:1, 0:128], rhs=rstdr2[:],
                                 start=True, stop=True)
                rstdb2 = efw.tile([128, CH], BF16, tag="rstdb2")
                nc.scalar.copy(out=rstdb2[:], in_=psb4[:])
                xc2 = ef.tile([128, KT, CH], BF16)
                for k in range(KT):
                    nc.vector.tensor_tensor(out=xc2[:, k, :], in0=x2[:, k, :],
                                            in1=mub2[:], op=ALU.subtract)
                h2t = ef.tile([128, KT, CH], F8)
                for k in range(KT):
                    nc.vector.tensor_tensor(out=h2t[:, k, :], in0=xc2[:, k, :],
                                            in1=rstdb2[:], op=ALU.mult)

                # FFN1 (fp8 DoubleRow), ReLU+bias+unscale fused on ACT
                hid = ef.tile([128, 32, CH], F8)
                for m in range(32):
                    w1t = w1p.tile([128, KT, 128], F8, tag="w1t")
                    nc.sync.dma_start(w1t[:], w1[:, :, 128 * m:128 * (m + 1)])
                    ps = psE.tile([128, CH], F32, tag="ps")
                    for t2 in range(KT // 2):
                        nc.tensor.matmul(ps[:], lhsT=w1t[:, 2 * t2:2 * t2 + 2, :],
                                         rhs=h2t[:, 2 * t2:2 * t2 + 2, :],
                                         start=(t2 == 0), stop=(t2 == KT // 2 - 1),
                                         perf_mode=DR)
                    nc.scalar.activation(out=hid[:, m, :], in_=ps[:], func=AF.Relu,
                                         bias=b1_sb[:, m:m + 1], scale=ISW)

                # FFN2 (fp8 DoubleRow) + b2 + residual
                for co in range(KT):
                    w2t = w2p.tile([128, 32, 128], F8, tag="w2t")
                    nc.sync.dma_start(w2t[:], w2[co])
                    ps = psE.tile([128, CH], F32, tag="ps")
                    for t2 in range(16):
                        nc.tensor.matmul(ps[:], lhsT=w2t[:, 2 * t2:2 * t2 + 2, :],
                                         rhs=hid[:, 2 * t2:2 * t2 + 2, :],
                                         start=(t2 == 0), stop=(t2 == 15),
                                         perf_mode=DR)
                    ft = efw.tile([128, CH], BF16, tag="ft")
                    nc.scalar.activation(out=ft[:], in_=ps[:], func=AF.Identity,
                                         bias=b2_sb[:, co:co + 1], scale=ISW2)
                    yt = efw.tile([128, CH], F32, tag="yt")
                    nc.vector.tensor_tensor(out=yt[:], in0=ft[:], in1=x2[:, co, :],
                                            op=ALU.add)
                    nc.sync.dma_start(y[co], yt[:])

    nc.compile()
    return nc


def prep_inputs(x, Wq, Wk, Wv, Wo, bo, W1, b1, W2, b2, g1, be1, g2, be2):
    """Host-side sharding / layout prep. Returns list of per-core input dicts."""
    bf = ml_dtypes.bfloat16
    f8 = mybir.dt.np(F8)
    x = np.asarray(x, np.float32)
    g1 = np.asarray(g1, np.float32); be1 = np.asarray(be1, np.float32)
    g2 = np.asarray(g2, np.float32); be2 = np.asarray(be2, np.float32)
    Wq = np.asarray(Wq, np.float32); Wk = np.asarray(Wk, np.float32)
    Wv = np.asarray(Wv, np.float32); Wo = np.asarray(Wo, np.float32)
    W1 = np.asarray(W1, np.float32); W2 = np.asarray(W2, np.float32)
    bo = np.asarray(bo, np.float32); b1 = np.asarray(b1, np.float32)
    b2 = np.asarray(b2, np.float32)

    Wq_f = g1[:, None] * Wq; bq_f = be1 @ Wq
    Wk_f = g1[:, None] * Wk; bk_f = be1 @ Wk
    Wv_f = g1[:, None] * Wv; bv_f = be1 @ Wv
    W1_f = g2[:, None] * W1; b1_f = b1 + be2 @ W1

    def lhsT_layout(w, scale):  # [C_in, M] -> [128, C_in//128, M] fp8
        ci, m = w.shape
        return np.ascontiguousarray(
            (w * scale).reshape(ci // 128, 128, m).transpose(1, 0, 2)).astype(f8)

    def tmajor(a):  # [rows, C] -> x^T tiles [128, KT, rows] bf16
        return np.ascontiguousarray(
            a.T.reshape(KT, 128, -1).transpose(1, 0, 2)).astype(bf)

    wo_l = np.ascontiguousarray(
        (Wo * SW).reshape(KT, 128, KT, 128).transpose(1, 0, 2, 3)).astype(f8)
    w1_l = lhsT_layout(W1_f, SW)
    w2_l = np.ascontiguousarray(
        (W2 * SW2).reshape(32, 128, KT, 128).transpose(2, 1, 0, 3)).astype(f8)
    b1_l = np.ascontiguousarray(b1_f.reshape(32, 128).T).astype(np.float32)
    b2_l = np.ascontiguousarray(b2.reshape(KT, 128).T).astype(np.float32)
    bo_l = np.ascontiguousarray(bo.reshape(KT, 128).T).astype(np.float32)

    # causal masks [p, d, hh, q]: valid iff 128*d + p <= q (hh dim replicated)
    m3 = np.zeros((128, 4, CH), np.float32)
    for d in range(4):
        kl = 128 * d + np.arange(128)[:, None]
        ql = np.arange(CH)[None, :]
        m3[:, d, :] = (kl <= ql).astype(np.float32)
    masks_l = np.ascontiguousarray(
        np.repeat(m3[:, :, None, :], 2, axis=2).transpose(0, 1, 2, 3)).astype(bf)

    ins = []
    for c in range(N_CORES):
        b = c // NG
        cols = slice(FPC * (c % NG), FPC * (c % NG + 1))
        xb = x[b]  # [T, C]
        strip = slice((CH // 2) * c, (CH // 2) * (c + 1))
        xown = np.concatenate([x[0][strip], x[1][strip]], axis=0)  # [CH, C]
        bq_c = bq_f[cols].reshape(2, 128).T
        bk_c = bk_f[cols].reshape(2, 128).T
        bv_c = bv_f[cols].reshape(2, 128).T
        ins.append({
            "xt": tmajor(xb),
            "xself": tmajor(xown),
            "wq": lhsT_layout(Wq_f[:, cols], SW),
            "wk": lhsT_layout(Wk_f[:, cols], SW),
            "wv": lhsT_layout(Wv_f[:, cols], SW),
            "bqkv": np.ascontiguousarray(
                np.stack([bq_c, bk_c, bv_c], axis=2)).astype(np.float32),
            "wo": wo_l, "bo_col": bo_l,
            "w1": w1_l, "b1": b1_l,
            "w2": w2_l, "b2col": b2_l,
            "masks": masks_l,
        })
    return ins


def postprocess(results):
    """Per-core y^T tiles [KT, 128, CH] -> full [B, T, C]."""
    out = np.empty((B, T, C), np.float32)
    hs = CH // 2
    for c, r in enumerate(results):
        yt = np.asarray(r["y"], np.float32).reshape(C, CH)  # [feats, toks]
        out[0, hs * c:hs * (c + 1), :] = yt[:, 0:hs].T
        out[1, hs * c:hs * (c + 1), :] = yt[:, hs:CH].T
    return out


_NC_CACHE = {}


def kernel(**inputs):
    import time
    from concourse.bass_utils import run_bass_kernel_spmd
    if "nc" not in _NC_CACHE:
        _NC_CACHE["nc"] = build_nc()
    nc = _NC_CACHE["nc"]
    ins = prep_inputs(**inputs)
    res = None
    last_exc = None
    for _attempt in range(4):
        try:
            res = run_bass_kernel_spmd(nc, ins, core_ids=list(range(N_CORES)))
            break
        except Exception as e:  # transient device wedge
            last_exc = e
            time.sleep(2)
    if res is None:
        raise last_exc
    return postprocess(res.results)


# revision 38
# speedup vs baseline: 1.1099x; 1.1099x over previous
"""Trainium2 Bass kernel for a dense transformer block (B=2, T=2048, C=1024, H=16).

Sharding v3: (batch, head-group) tensor-parallel attention across 8 cores
(core = one batch x 4 heads), 8-rank AllToAll with cross-batch 256-token
strips, then row-parallel FFN (512 tokens/core). Feature-major dataflow off
a host-transposed x^T; LN stats via ones-matmuls; fp8 DoubleRow matmuls for
QKV / Wo / FFN with host-scaled weights; fp8 A2A payload. Output y^T is
un-transposed on the host.
"""

import numpy as np
import ml_dtypes

import concourse.bass as bass
import concourse.bacc as bacc
import concourse.mybir as mybir
import concourse.tile as tile
from concourse.masks import make_identity


F32 = mybir.dt.float32
BF16 = mybir.dt.bfloat16
F8 = mybir.dt.float8e4
AF = mybir.ActivationFunctionType
ALU = mybir.AluOpType
DR = mybir.MatmulPerfMode.DoubleRow

N_CORES = 8
NG = 4                  # cores per group (one batch per group)
B, T, C, H, D, FF = 2, 2048, 1024, 16, 64, 4096
HPC = H // NG           # 4 heads per core
FPC = HPC * D           # 256 features per core
KT = C // 128           # 8 k-tiles of embedding dim
CH = 512                # token chunk
NCH = T // CH           # 4 chunks per batch
SCALE = 1.0 / np.sqrt(C)
LN_EPS = 1e-5
SW = 2.0 ** 12          # fp8 weight scale (wq/wk/wv/wo/w1)
SW2 = 2.0 ** 13         # fp8 weight scale (w2)
ISW = 1.0 / SW
ISW2 = 1.0 / SW2
N_WARM = 36
N_DUMMY = 44


def build_nc():
    nc = bacc.Bacc(None, target_bir_lowering=False, debug=False, num_devices=N_CORES)

    # ---- per-core inputs (host pre-laid-out) ----
    xt = nc.dram_tensor("xt", [128, KT, T], BF16, kind="ExternalInput").ap()
    xself = nc.dram_tensor("xself", [128, KT, CH], BF16, kind="ExternalInput").ap()
    wq = nc.dram_tensor("wq", [128, KT, FPC], F8, kind="ExternalInput").ap()
    wk = nc.dram_tensor("wk", [128, KT, FPC], F8, kind="ExternalInput").ap()
    wv = nc.dram_tensor("wv", [128, KT, FPC], F8, kind="ExternalInput").ap()
    bqkv = nc.dram_tensor("bqkv", [128, 2, 3], F32, kind="ExternalInput").ap()
    wo = nc.dram_tensor("wo", [128, KT, KT, 128], F8, kind="ExternalInput").ap()
    bo_col = nc.dram_tensor("bo_col", [128, KT], F32, kind="ExternalInput").ap()
    w1 = nc.dram_tensor("w1", [128, KT, FF], F8, kind="ExternalInput").ap()
    b1 = nc.dram_tensor("b1", [128, 32], F32, kind="ExternalInput").ap()
    w2 = nc.dram_tensor("w2", [KT, 128, 32, 128], F8, kind="ExternalInput").ap()
    b2col = nc.dram_tensor("b2col", [128, KT], F32, kind="ExternalInput").ap()
    masks = nc.dram_tensor("masks", [128, 4, 2, CH], BF16, kind="ExternalInput").ap()
    y = nc.dram_tensor("y", [KT, 128, CH], F32, kind="ExternalOutput").ap()

    with tile.TileContext(nc) as tc:
        with (
            tc.tile_pool(name="const", bufs=1) as const,
            tc.tile_pool(name="dram", bufs=1, space="DRAM") as dram,
        ):
            ident = const.tile([128, 128], BF16)
            make_identity(nc, ident[:])
            ones_c = const.tile([128, 1], BF16)
            nc.any.memset(ones_c[:], 1.0 / C)
            ones_1 = const.tile([128, 1], BF16)
            nc.any.memset(ones_1[:], 1.0)
            ones64b = const.tile([128, 64], BF16)
            nc.any.memset(ones64b[:], 1.0)
            onesrow = const.tile([1, CH], BF16)
            nc.any.memset(onesrow[:], 1.0)
            eps1 = const.tile([1, 1], F32)
            nc.any.memset(eps1[:], LN_EPS)

            # A2A slots: dest core c' gets my 4 heads for a 256-token strip of
            # my batch (stage E rows: 256 from b0 + 256 from b1)
            a2a_in = dram.tile([N_CORES, 2, 128, CH // 2], F8)
            a2a_out = dram.tile([N_CORES, 2, 128, CH // 2], F8)

            # attention persistent tensors
            qkv_cm = tc.tile_pool(name="qkvp", bufs=1)
            qkvp = qkv_cm.__enter__()
            qt_sb = qkvp.tile([128, 2, T], BF16)
            kt_sb = qkvp.tile([128, 2, T], BF16)
            vt_sb = qkvp.tile([128, 2, T], BF16)
            v_sb = qkvp.tile([128, T // 128, FPC], BF16)
            masks_sb = qkvp.tile([128, 4, 2, CH], BF16)

            # ================= Phase 1: LN1 + QKV (feature-major) =================
            with (
                tc.tile_pool(name="p1", bufs=2) as p1,
                tc.tile_pool(name="p1s", bufs=2) as p1s,
                tc.tile_pool(name="ps1", bufs=3, space="PSUM") as ps1,
                tc.tile_pool(name="pstat", bufs=2, space="PSUM") as pstat,
            ):
                # x^T chunk loads first so the stats matmuls can start early
                xts = []
                for n in range(NCH):
                    xt_c = p1.tile([128, KT, CH], BF16, tag="xt", bufs=4,
                                   name=f"xtc{n}")
                    nc.sync.dma_start(xt_c[:], xt[:, :, CH * n:CH * (n + 1)])
                    xts.append(xt_c)
                # weights after the x^T stream
                wq_sb = const.tile([128, KT, FPC], F8, name="wq_sb")
                nc.sync.dma_start(wq_sb[:], wq[:])
                wk_sb = const.tile([128, KT, FPC], F8, name="wk_sb")
                nc.sync.dma_start(wk_sb[:], wk[:])
                wv_sb = const.tile([128, KT, FPC], F8, name="wv_sb")
                nc.sync.dma_start(wv_sb[:], wv[:])
                bqkv_sb = const.tile([128, 2, 3], F32, name="bqkv_sb")
                nc.sync.dma_start(bqkv_sb[:], bqkv[:])
                nc.sync.dma_start(masks_sb[:], masks[:])
                xself_sb = const.tile([128, KT, CH], BF16, name="xself_sb")
                nc.sync.dma_start(xself_sb[:], xself[:])
                wo_sb = const.tile([128, KT, KT, 128], F8, name="wo_sb")
                bo_sb = const.tile([128, KT], F32, name="bo_sb")
                nc.sync.dma_start(bo_sb[:], bo_col[:])
                b1_sb = const.tile([128, 32], F32, name="b1_sb")
                nc.sync.dma_start(b1_sb[:], b1[:])
                b2_sb = const.tile([128, KT], F32, name="b2_sb")
                nc.sync.dma_start(b2_sb[:], b2col[:])

                # HAM warmup: PE activity with no DMA dependency
                ps_w = ps1.tile([128, CH], F32, tag="warm", bufs=1)
                for wi in range(N_WARM):
                    nc.tensor.matmul(ps_w[:, 0:128], lhsT=ident[:], rhs=ident[:],
                                     start=(wi == 0), stop=(wi == N_WARM - 1))

                for n in range(NCH):
                    q0 = CH * n
                    xt_c = xts[n]
                    st = pstat.tile([128, CH], F32, tag="stat")
                    for k in range(KT):
                        nc.tensor.matmul(st[0:1, :], lhsT=ones_c[:], rhs=xt_c[:, k, :],
                                         start=(k == 0), stop=(k == KT - 1))
                    # E[x^2] from raw x (independent of mu -> shorter chain)
                    for k in range(KT):
                        sq = p1s.tile([128, CH], BF16, tag="sq", bufs=3)
                        nc.vector.tensor_tensor(out=sq[:], in0=xt_c[:, k, :],
                                                in1=xt_c[:, k, :], op=ALU.mult)
                        nc.tensor.matmul(st[32:33, :], lhsT=ones_c[:], rhs=sq[:],
                                         start=(k == 0), stop=(k == KT - 1))
                    mur = p1s.tile([1, CH], BF16, tag="mur")
                    nc.scalar.copy(out=mur[:], in_=st[0:1, :])
                    murf = p1s.tile([1, CH], F32, tag="murf")
                    nc.scalar.copy(out=murf[:], in_=st[0:1, :])
                    psb = ps1.tile([128, CH], F32, tag="psmm")
                    nc.tensor.matmul(psb[:], lhsT=onesrow[0:1, 0:128], rhs=mur[:],
                                     start=True, stop=True)
                    mub = p1s.tile([128, CH], BF16, tag="mub")
                    nc.scalar.copy(out=mub[:], in_=psb[:])
                    musq = p1s.tile([1, CH], F32, tag="musq")
                    nc.vector.tensor_tensor(out=musq[:], in0=murf[:], in1=murf[:],
                                            op=ALU.mult)
                    varr = p1s.tile([1, CH], F32, tag="varr")
                    nc.vector.tensor_tensor(out=varr[:], in0=st[32:33, :], in1=musq[:],
                                            op=ALU.subtract)
                    stdr = p1s.tile([1, CH], F32, tag="stdr")
                    nc.scalar.activation(out=stdr[:], in_=varr[:], func=AF.Sqrt,
                                         bias=eps1[:], scale=1.0)
                    rstdr = p1s.tile([1, CH], BF16, tag="rstdr")
                    with nc.allow_low_precision(reason="ln rstd bf16"):
                        nc.vector.reciprocal(out=rstdr[:], in_=stdr[:])
                    psb2 = ps1.tile([128, CH], F32, tag="psmm")
                    nc.tensor.matmul(psb2[:], lhsT=onesrow[0:1, 0:128], rhs=rstdr[:],
                                     start=True, stop=True)
                    rstdb = p1s.tile([128, CH], BF16, tag="rstdb")
                    nc.scalar.copy(out=rstdb[:], in_=psb2[:])
                    xc = p1.tile([128, KT, CH], BF16, tag="xc")
                    for k in range(KT):
                        nc.vector.tensor_tensor(out=xc[:, k, :], in0=xt_c[:, k, :],
                                                in1=mub[:], op=ALU.subtract)
                    h_c = p1.tile([128, KT, CH], F8, tag="h")
                    for k in range(KT):
                        nc.vector.tensor_tensor(out=h_c[:, k, :], in0=xc[:, k, :],
                                                in1=rstdb[:], op=ALU.mult)
                    # QKV matmuls: fp8 DoubleRow, weights pre-scaled by SW
                    for w_sb, out_sb, col in ((wq_sb, qt_sb, 0), (wk_sb, kt_sb, 1),
                                              (wv_sb, vt_sb, 2)):
                        for g in range(2):
                            ps = ps1.tile([128, CH], F32, tag="psmm")
                            for t2 in range(KT // 2):
                                nc.tensor.matmul(
                                    ps[:],
                                    lhsT=w_sb[:, 2 * t2:2 * t2 + 2, 128 * g:128 * (g + 1)],
                                    rhs=h_c[:, 2 * t2:2 * t2 + 2, :],
                                    start=(t2 == 0), stop=(t2 == KT // 2 - 1),
                                    perf_mode=DR)
                            nc.vector.tensor_scalar(
                                out=out_sb[:, g, q0:q0 + CH], in0=ps[:],
                                scalar1=ISW, scalar2=bqkv_sb[:, g, col:col + 1],
                                op0=ALU.mult, op1=ALU.add)
                    # V -> token-major for this chunk (PE transposes)
                    for g in range(2):
                        ps_t = ps1.tile([128, CH], BF16, tag="psmm")
                        for u in range(4):
                            nc.tensor.transpose(
                                ps_t[:, 128 * u:128 * (u + 1)],
                                vt_sb[:, g, q0 + 128 * u:q0 + 128 * (u + 1)], ident[:])
                        nc.scalar.copy(
                            out=v_sb[:, 4 * n:4 * n + 4, 128 * g:128 * (g + 1)],
                            in_=ps_t[:].rearrange("p (a b) -> p a b", a=4))
                nc.sync.dma_start(wo_sb[:], wo[:])

            # ================= Phase 2: attention (S^T orientation) =================
            with (
                tc.tile_pool(name="pss", bufs=1, space="PSUM") as pssp,
                tc.tile_pool(name="pap", bufs=1, space="PSUM") as pap,
                tc.tile_pool(name="psr", bufs=1, space="PSUM") as psr,
                tc.tile_pool(name="ptp", bufs=9) as ptp,
                tc.tile_pool(name="smp", bufs=2) as smp,
            ):
                def make_evac(qc, pa, pasum):
                    # deferred normalize+ship of a finished q-chunk; emitted
                    # after the NEXT chunk's first QK matmuls so the rb
                    # broadcast matmuls (gated on the DVE reciprocal) do not
                    # head-of-line-block the PE queue
                    def evac():
                        recf = smp.tile([128, CH], F32, tag="recf", name="recf")
                        nc.vector.reciprocal_approx_fast(out=recf[:], in_=pasum[:])
                        rec = smp.tile([128, CH], BF16, tag="rec", name="rec")
                        nc.vector.tensor_scalar(out=rec[:], in0=recf[:], scalar1=1.0,
                                                scalar2=None, op0=ALU.mult)
                        for gg in range(2):
                            an = smp.tile([128, CH], BF16, tag=f"an{gg}", name="an")
                            nc.scalar.copy(out=an[:], in_=pa[gg][:])
                            rb = psr.tile([128, CH], F32, tag="recb", name="rb")
                            for hh in range(2):
                                h = 2 * gg + hh
                                nc.tensor.matmul(
                                    rb[64 * hh:64 * (hh + 1), :],
                                    lhsT=ones64b[32 * h:32 * h + 1, :],
                                    rhs=rec[32 * h:32 * h + 1, :],
                                    start=True, stop=True,
                                    tile_position=(32 * h, 64 * hh),
                                    skip_group_check=(hh == 1))
                            at_t = smp.tile([128, CH], F8, tag=f"at{gg}", name="at_t")
                            nc.vector.tensor_tensor(out=at_t[:], in0=an[:], in1=rb[:],
                                                    op=ALU.mult)
                            nc.sync.dma_start(out=a2a_in[2 * qc, gg],
                                              in_=at_t[:, 0:CH // 2])
                            nc.sync.dma_start(out=a2a_in[2 * qc + 1, gg],
                                              in_=at_t[:, CH // 2:CH])
                    return evac

                pending_evac = None
                for qc in range(NCH):
                    q0 = CH * qc
                    nkt = 4 * (qc + 1)
                    pa0 = pap.tile([128, CH], F32, tag="pa0", name="pa0")
                    pa1 = pap.tile([128, CH], F32, tag="pa1", name="pa1")
                    pa = [pa0, pa1]
                    pasum = pap.tile([128, CH], F32, tag="pasum")
                    pts = {}

                    def emit_qk(k):
                        d = k - 4 * qc
                        qlo = 128 * d if d > 0 else 0
                        for gg in range(2):
                            ps = pssp.tile([128, 2, CH], F32, tag=f"pss{gg}")
                            for hh in range(2):
                                hp = 64 * hh
                                nc.tensor.matmul(
                                    ps[:, hh, qlo:],
                                    lhsT=kt_sb[hp:hp + 64, gg, 128 * k:128 * (k + 1)],
                                    rhs=qt_sb[hp:hp + 64, gg, q0 + qlo:q0 + CH],
                                    start=True, stop=True, tile_position=(hp, 0))
                            pt = ptp.tile([128, 2, CH], BF16, tag="pt")
                            nc.scalar.activation(out=pt[:, :, qlo:], in_=ps[:, :, qlo:],
                                                 func=AF.Exp, scale=SCALE)
                            if d >= 0:
                                nc.gpsimd.tensor_tensor(
                                    out=pt[:, :, qlo:], in0=pt[:, :, qlo:],
                                    in1=masks_sb[:, d, :, qlo:], op=ALU.mult)
                            pts[(k, gg)] = pt

                    def emit_pv(k):
                        d = k - 4 * qc
                        qlo = 128 * d if d > 0 else 0
                        for gg in range(2):
                            pt = pts.pop((k, gg))
                            for hh in range(2):
                                nc.tensor.matmul(
                                    pa[gg][64 * hh:64 * (hh + 1), qlo:],
                                    lhsT=v_sb[:, k, 128 * gg + 64 * hh:128 * gg + 64 * (hh + 1)],
                                    rhs=pt[:, hh, qlo:],
                                    start=(k == 0), stop=(k == nkt - 1),
                                    tile_position=(0, 64 * hh),
                                    skip_group_check=(hh == 1))
                            for hh in range(2):
                                h = 2 * gg + hh
                                nc.tensor.matmul(
                                    pasum[32 * h:32 * h + 1, qlo:],
                                    lhsT=ones_1[:], rhs=pt[:, hh, qlo:],
                                    start=(k == 0), stop=(k == nkt - 1),
                                    tile_position=(0, 32 * h),
                                    skip_group_check=(h > 0))

                    for k in range(nkt + 3):
                        if k < nkt:
                            emit_qk(k)
                        if k == 2 and pending_evac is not None:
                            pending_evac()
                            pending_evac = None
                        if k >= 3:
                            emit_pv(k - 3)
                    pending_evac = make_evac(qc, pa, pasum)
                pending_evac()

            qkv_cm.__exit__(None, None, None)

            # ================= Phase 3: AllToAll (8 ranks, fp8 payload) ============
            nc.gpsimd.collective_compute(
                "AllToAll", ALU.bypass,
                replica_groups=[list(range(N_CORES))],
                ins=[a2a_in[:].opt()], outs=[a2a_out[:].opt()],
            )

            # ================= Phase 4: Wo + LN2 + FFN (feature-major) ============
            with (
                tc.tile_pool(name="ef", bufs=1) as ef,
                tc.tile_pool(name="efw", bufs=2) as efw,
                tc.tile_pool(name="psE", bufs=3, space="PSUM") as psE,
                tc.tile_pool(name="psES", bufs=1, space="PSUM") as psES,
                tc.tile_pool(name="w1p", bufs=4) as w1p,
                tc.tile_pool(name="w2p", bufs=3) as w2p,
            ):
                # keep-warm dummies riding over the collective
                dm = psES.tile([128, CH], F32, tag="dummy")
                for i in range(N_DUMMY):
                    nc.tensor.matmul(dm[:], lhsT=ident[:],
                                     rhs=xself_sb[:, 0, :],
                                     start=(i == 0), stop=(i == N_DUMMY - 1))

                # token axis of stage E: [0:256] = batch-0 strip, [256:512] = batch-1
                attnt = ef.tile([128, KT, CH], F8)
                for s in range(N_CORES):
                    bs, hgs = s // NG, s % NG
                    for g in range(2):
                        nc.sync.dma_start(
                            out=attnt[:, 2 * hgs + g,
                                      (CH // 2) * bs:(CH // 2) * (bs + 1)],
                            in_=a2a_out[s, g])

                # Wo (fp8 DoubleRow) + bo + residual
                x2 = ef.tile([128, KT, CH], BF16)
                for co in range(KT):
                    ps = psE.tile([128, CH], F32, tag="ps")
                    for t2 in range(KT // 2):
                        nc.tensor.matmul(ps[:],
                                         lhsT=wo_sb[:, 2 * t2:2 * t2 + 2, co, :],
                                         rhs=attnt[:, 2 * t2:2 * t2 + 2, :],
                                         start=(t2 == 0), stop=(t2 == KT // 2 - 1),
                                         perf_mode=DR)
                    prj = efw.tile([128, CH], BF16, tag="prj")
                    nc.scalar.activation(out=prj[:], in_=ps[:], func=AF.Identity,
                                         bias=bo_sb[:, co:co + 1], scale=ISW)
                    nc.vector.tensor_tensor(out=x2[:, co, :], in0=prj[:],
                                            in1=xself_sb[:, co, :], op=ALU.add)

                # LN2 (feature-major stats, E[x^2] form)
                st2 = psES.tile([128, CH], F32, tag="stat2")
                for k in range(KT):
                    nc.tensor.matmul(st2[0:1, :], lhsT=ones_c[:], rhs=x2[:, k, :],
                                     start=(k == 0), stop=(k == KT - 1))
                for k in range(KT):
                    sq2 = efw.tile([128, CH], BF16, tag="sq2", bufs=3)
                    nc.vector.tensor_tensor(out=sq2[:], in0=x2[:, k, :],
                                            in1=x2[:, k, :], op=ALU.mult)
                    nc.tensor.matmul(st2[32:33, :], lhsT=ones_c[:], rhs=sq2[:],
                                     start=(k == 0), stop=(k == KT - 1))
                mur2 = efw.tile([1, CH], BF16, tag="mur2")
                nc.scalar.copy(out=mur2[:], in_=st2[0:1, :])
                murf2 = efw.tile([1, CH], F32, tag="murf2")
                nc.scalar.copy(out=murf2[:], in_=st2[0:1, :])
                psb3 = psE.tile([128, CH], F32, tag="ps")
                nc.tensor.matmul(psb3[:], lhsT=onesrow[0:1, 0:128], rhs=mur2[:],
                                 start=True, stop=True)
                mub2 = efw.tile([128, CH], BF16, tag="mub2")
                nc.scalar.copy(out=mub2[:], in_=psb3[:])
                musq2 = efw.tile([1, CH], F32, tag="musq2")
                nc.vector.tensor_tensor(out=musq2[:], in0=murf2[:], in1=murf2[:],
                                        op=ALU.mult)
                varr2 = efw.tile([1, CH], F32, tag="varr2")
                nc.vector.tensor_tensor(out=varr2[:], in0=st2[32:33, :], in1=musq2[:],
                                        op=ALU.subtract)
                stdr2 = efw.tile([1, CH], F32, tag="stdr2")
                nc.scalar.activation(out=stdr2[:], in_=varr2[:], func=AF.Sqrt,
                                     bias=eps1[:], scale=1.0)
                rstdr2 = efw.tile([1, CH], BF16, tag="rstdr2")
                with nc.allow_low_precision(reason="ln2 rstd bf16"):
                    nc.vector.reciprocal(out=rstdr2[:], in_=stdr2[:])
                psb4 = psE.tile([128, CH], F32, tag="ps")
                nc.tensor.matmul(psb4[:], lhsT=onesrow[0:1, 0:128], rhs=rstdr2[:],
                                 start=True, stop=True)
                rstdb2 = efw.tile([128, CH], BF16, tag="rstdb2")
                nc.scalar.copy(out=rstdb2[:], in_=psb4[:])
                xc2 = ef.tile([128, KT, CH], BF16)
                for k in range(KT):
                    nc.vector.tensor_tensor(out=xc2[:, k, :], in0=x2[:, k, :],
                                            in1=mub2[:], op=ALU.subtract)
                h2t = ef.tile([128, KT, CH], F8)
                for k in range(KT):
                    nc.vector.tensor_tensor(out=h2t[:, k, :], in0=xc2[:, k, :],
                                            in1=rstdb2[:], op=ALU.mult)

                # FFN1 (fp8 DoubleRow), ReLU+bias+unscale fused on ACT
                hid = ef.tile([128, 32, CH], F8)
                for m in range(32):
                    w1t = w1p.tile([128, KT, 128], F8, tag="w1t")
                    nc.sync.dma_start(w1t[:], w1[:, :, 128 * m:128 * (m + 1)])
                    ps = psE.tile([128, CH], F32, tag="ps")
                    for t2 in range(KT // 2):
                        nc.tensor.matmul(ps[:], lhsT=w1t[:, 2 * t2:2 * t2 + 2, :],
                                         rhs=h2t[:, 2 * t2:2 * t2 + 2, :],
                                         start=(t2 == 0), stop=(t2 == KT // 2 - 1),
                                         perf_mode=DR)
                    nc.scalar.activation(out=hid[:, m, :], in_=ps[:], func=AF.Relu,
                                         bias=b1_sb[:, m:m + 1], scale=ISW)

                # FFN2 (fp8 DoubleRow) + b2 + residual
                for co in range(KT):
                    w2t = w2p.tile([128, 32, 128], F8, tag="w2t")
                    nc.sync.dma_start(w2t[:], w2[co])
                    ps = psE.tile([128, CH], F32, tag="ps")
                    for t2 in range(16):
                        nc.tensor.matmul(ps[:], lhsT=w2t[:, 2 * t2:2 * t2 + 2, :],
                                         rhs=hid[:, 2 * t2:2 * t2 + 2, :],
                                         start=(t2 == 0), stop=(t2 == 15),
                                         perf_mode=DR)
                    ft = efw.tile([128, CH], BF16, tag="ft")
                    nc.scalar.activation(out=ft[:], in_=ps[:], func=AF.Identity,
                                         bias=b2_sb[:, co:co + 1], scale=ISW2)
                    yt = efw.tile([128, CH], F32, tag="yt")
                    nc.vector.tensor_tensor(out=yt[:], in0=ft[:], in1=x2[:, co, :],
                                            op=ALU.add)
                    nc.sync.dma_start(y[co], yt[:])

    nc.compile()
    return nc


def prep_inputs(x, Wq, Wk, Wv, Wo, bo, W1, b1, W2, b2, g1, be1, g2, be2):
    """Host-side sharding / layout prep. Returns list of per-core input dicts."""
    bf = ml_dtypes.bfloat16
    f8 = mybir.dt.np(F8)
    x = np.asarray(x, np.float32)
    g1 = np.asarray(g1, np.float32); be1 = np.asarray(be1, np.float32)
    g2 = np.asarray(g2, np.float32); be2 = np.asarray(be2, np.float32)
    Wq = np.asarray(Wq, np.float32); Wk = np.asarray(Wk, np.float32)
    Wv = np.asarray(Wv, np.float32); Wo = np.asarray(Wo, np.float32)
    W1 = np.asarray(W1, np.float32); W2 = np.asarray(W2, np.float32)
    bo = np.asarray(bo, np.float32); b1 = np.asarray(b1, np.float32)
    b2 = np.asarray(b2, np.float32)

    Wq_f = g1[:, None] * Wq; bq_f = be1 @ Wq
    Wk_f = g1[:, None] * Wk; bk_f = be1 @ Wk
    Wv_f = g1[:, None] * Wv; bv_f = be1 @ Wv
    W1_f = g2[:, None] * W1; b1_f = b1 + be2 @ W1

    def lhsT_layout(w, scale):  # [C_in, M] -> [128, C_in//128, M] fp8
        ci, m = w.shape
        return np.ascontiguousarray(
            (w * scale).reshape(ci // 128, 128, m).transpose(1, 0, 2)).astype(f8)

    def tmajor(a):  # [rows, C] -> x^T tiles [128, KT, rows] bf16
        return np.ascontiguousarray(
            a.T.reshape(KT, 128, -1).transpose(1, 0, 2)).astype(bf)

    wo_l = np.ascontiguousarray(
        (Wo * SW).reshape(KT, 128, KT, 128).transpose(1, 0, 2, 3)).astype(f8)
    w1_l = lhsT_layout(W1_f, SW)
    w2_l = np.ascontiguousarray(
        (W2 * SW2).reshape(32, 128, KT, 128).transpose(2, 1, 0, 3)).astype(f8)
    b1_l = np.ascontiguousarray(b1_f.reshape(32, 128).T).astype(np.float32)
    b2_l = np.ascontiguousarray(b2.reshape(KT, 128).T).astype(np.float32)
    bo_l = np.ascontiguousarray(bo.reshape(KT, 128).T).astype(np.float32)

    # causal masks [p, d, hh, q]: valid iff 128*d + p <= q (hh dim replicated)
    m3 = np.zeros((128, 4, CH), np.float32)
    for d in range(4):
        kl = 128 * d + np.arange(128)[:, None]
        ql = np.arange(CH)[None, :]
        m3[:, d, :] = (kl <= ql).astype(np.float32)
    masks_l = np.ascontiguousarray(
        np.repeat(m3[:, :, None, :], 2, axis=2).transpose(0, 1, 2, 3)).astype(bf)

    ins = []
    for c in range(N_CORES):
        b = c // NG
        cols = slice(FPC * (c % NG), FPC * (c % NG + 1))
        xb = x[b]  # [T, C]
        strip = slice((CH // 2) * c, (CH // 2) * (c + 1))
        xown = np.concatenate([x[0][strip], x[1][strip]], axis=0)  # [CH, C]
        bq_c = bq_f[cols].reshape(2, 128).T
        bk_c = bk_f[cols].reshape(2, 128).T
        bv_c = bv_f[cols].reshape(2, 128).T
        ins.append({
            "xt": tmajor(xb),
            "xself": tmajor(xown),
            "wq": lhsT_layout(Wq_f[:, cols], SW),
            "wk": lhsT_layout(Wk_f[:, cols], SW),
            "wv": lhsT_layout(Wv_f[:, cols], SW),
            "bqkv": np.ascontiguousarray(
                np.stack([bq_c, bk_c, bv_c], axis=2)).astype(np.float32),
            "wo": wo_l, "bo_col": bo_l,
            "w1": w1_l, "b1": b1_l,
            "w2": w2_l, "b2col": b2_l,
            "masks": masks_l,
        })
    return ins


def postprocess(results):
    """Per-core y^T tiles [KT, 128, CH] -> full [B, T, C]."""
    out = np.empty((B, T, C), np.float32)
    hs = CH // 2
    for c, r in enumerate(results):
        yt = np.asarray(r["y"], np.float32).reshape(C, CH)  # [feats, toks]
        out[0, hs * c:hs * (c + 1), :] = yt[:, 0:hs].T
        out[1, hs * c:hs * (c + 1), :] = yt[:, hs:CH].T
    return out


_NC_CACHE = {}


def kernel(**inputs):
    import time
    from concourse.bass_utils import run_bass_kernel_spmd
    if "nc" not in _NC_CACHE:
        _NC_CACHE["nc"] = build_nc()
    nc = _NC_CACHE["nc"]
    ins = prep_inputs(**inputs)
    res = None
    last_exc = None
    for _attempt in range(4):
        try:
            res = run_bass_kernel_spmd(nc, ins, core_ids=list(range(N_CORES)))
            break
        except Exception as e:  # transient device wedge
            last_exc = e
            time.sleep(2)
    if res is None:
        raise last_exc
    return postprocess(res.results)
